# revision 19
# baseline (speedup 1.0000x reference)
"""Cost-sensitive cross-entropy loss on 8 Trainium2 NeuronCores.

Strategy (data-parallel over batch):
  - Each of the 8 cores processes a 16384-row shard of `outputs` [131072, 1000].
  - Per 128-row tile the device computes: row-wise sum(exp(x)) (ScalarE Exp with
    accumulate) and row-wise argmax (VectorE max -> max_index, first-occurrence
    semantics matching jnp.argmax).  No max-subtraction is needed: inputs are
    O(1)-scaled so exp() cannot overflow fp32, and skipping it decouples ACT
    from DVE for full engine overlap.
  - Host combines: lse = log(sumexp), glp = x[i,t_i] - lse (gather), scatter-add
    count matrix from (target, predicted), cost-matrix transform, final scalar.
    All host math is O(B + C^2) ~ 1e6 elements vs the 5e8-element device pass.
"""

import contextlib
import sys

import numpy as np

try:
    import concourse.bass as bass
except ImportError:  # stand-alone grading dir: fall back to the repo install
    for p in ("/opt/trn_rl_repo", "/root/.axon_site/_ro/trn_rl_repo"):
        if p not in sys.path:
            sys.path.insert(0, p)
    import concourse.bass as bass

import concourse.mybir as mybir
from concourse import bass_utils

B, C = 131072, 1000
NCORES = 8
SHARD = B // NCORES  # 16384 rows per core
P = 128              # SBUF partitions = rows per tile
NT = SHARD // P      # 128 tiles per core
BETA1, BETA2 = 1.0, 2.0

_nc_cache = {}


def build_bass(nt: int = NT, sx: int = 5, se: int = 8, tpl: int = 4):
    """One-core program; SPMD-replicated across all 8 cores.

    Raw Bass (no Tile): this walrus build allows at most one embedded sync
    wait per instruction, so all waits are standalone wait_ge instructions
    on the issuing engine's stream.

    Loads carry `tpl` 128-row tiles each (2 MB for tpl=4) to amortize the
    ~0.8 us per-DMA-instruction bubble observed on every SDMA engine.

    Pipeline per load L / tile t:
      SP:  [wait exps of slot L-sx done]  dma xt[L%sx] <- x rows
      ACT: [wait load landed; wait argmax(t-se) done]
           ex[t%se] = exp(xt slice), accum -> stage_s[:, t]   (+1 act_sem)
      DVE: [wait exp t done] max8(t) ... max_index(t-2)       (+1 dve_sem)
    """
    key = (nt, sx, se, tpl)
    if key in _nc_cache:
        return _nc_cache[key]

    f32 = mybir.dt.float32
    u32 = mybir.dt.uint32
    Exp = mybir.ActivationFunctionType.Exp
    assert nt % tpl == 0
    nl = nt // tpl  # number of loads

    nc = bass.Bass()
    x = nc.declare_dram_parameter("x", [nt * P, C], f32, isOutput=False)
    s_out = nc.declare_dram_parameter("s_out", [P, nt], f32, isOutput=True)
    i_out = nc.declare_dram_parameter("i_out", [P, nt * 8], u32, isOutput=True)

    with contextlib.ExitStack() as ctx:
        xt = [ctx.enter_context(nc.sbuf_tensor(f"xt{i}", [P, tpl * C], f32))
              for i in range(sx)]
        ex = [ctx.enter_context(nc.sbuf_tensor(f"ex{i}", [P, C], f32))
              for i in range(se)]
        mx8 = [ctx.enter_context(nc.sbuf_tensor(f"mx8_{i}", [P, 8], f32))
               for i in range(4)]
        stage_s = ctx.enter_context(nc.sbuf_tensor("stage_s", [P, nt], f32))
        stage_i = ctx.enter_context(
            nc.sbuf_tensor("stage_i", [P, nt * 8], u32))
        dma_sem = [ctx.enter_context(nc.semaphore(f"dma_sem{i}"))
                   for i in range(sx)]
        out_sem = ctx.enter_context(nc.semaphore("out_sem"))
        act_sem = ctx.enter_context(nc.semaphore("act_sem"))
        dve_sem = ctx.enter_context(nc.semaphore("dve_sem"))
        vsem = ctx.enter_context(nc.semaphore("vsem"))
        block = ctx.enter_context(nc.Block())

        def load(eng, L):
            eng.dma_start(
                out=xt[L % sx][:].rearrange("p (j c) -> p j c", j=tpl),
                in_=x[L * tpl * P:(L + 1) * tpl * P, :].rearrange(
                    "(j p) c -> p j c", p=P),
            ).then_inc(dma_sem[L % sx], 16)

        # One HWDGE queue sustains only ~250 GB/s of descriptor generation,
        # so loads alternate between the SP queue (even L) and the ACT queue
        # (odd L, trigger embedded in the exp stream at a point where its
        # guard wait is already satisfied -> no stall).
        @block.sync
        def _(sync):
            for L in range(0, nl, 2):
                if L >= sx:
                    # all tpl exps of slot L-sx done -> slot free
                    sync.wait_ge(act_sem, tpl * (L - sx + 1))
                load(sync, L)
            sync.wait_ge(act_sem, nt)
            sync.dma_start(out=s_out[:, :], in_=stage_s[:]).then_inc(out_sem, 16)
            sync.wait_ge(dve_sem, nt)
            sync.dma_start(out=i_out[:, :], in_=stage_i[:]).then_inc(out_sem, 16)
            sync.wait_ge(out_sem, 32)

        @block.scalar
        def _(scalar):
            # odd load L fires after exp tile tpl*(L-sx+1)+1: its guard
            # act_sem >= tpl*(L-sx+1) completed >=2 exps ago (no stall), and
            # the data isn't needed until tile L*tpl, ~tpl*(sx-1) tiles away.
            trig = {}
            for L in range(1, nl, 2):
                t_fire = max(tpl * (L - sx + 1) + 1, 0)
                trig.setdefault(t_fire, []).append(L)
            for L in trig.get(0, []):
                load(scalar, L)
            for t in range(nt):
                L, j = divmod(t, tpl)
                if j == 0:
                    scalar.wait_ge(dma_sem[L % sx], 16 * (L // sx + 1))
                if t >= se:
                    # argmax(t-se) read ex slot t%se -> slot free
                    scalar.wait_ge(dve_sem, t - se + 1)
                scalar.activation(
                    ex[t % se][:], xt[L % sx][:, j * C:(j + 1) * C], Exp,
                    accum_out=stage_s[:, t:t + 1],
                ).then_inc(act_sem, 1)
                for Lq in trig.get(t + 1, []):
                    scalar.wait_ge(act_sem, tpl * (Lq - sx + 1))
                    load(scalar, Lq)

        @block.vector
        def _(vector):
            # 2-stage SW pipeline: max(t) runs 2 tiles ahead of
            # max_index(t-2), so every sem wait (RAW on mx8, WAR on mx8
            # slot recycle) is satisfied well before it executes -- no DVE
            # pipeline drain.
            def mi(t):
                vector.wait_ge(vsem, t + 1)
                vector.max_index(
                    stage_i[:, t * 8:(t + 1) * 8], mx8[t % 4][:],
                    ex[t % se][:],
                ).then_inc(dve_sem, 1)

            for t in range(nt):
                if t >= 4:
                    vector.wait_ge(dve_sem, t - 3)  # mi(t-4) freed mx8 slot
                vector.wait_ge(act_sem, t + 1)
                vector.max(mx8[t % 4][:], ex[t % se][:]).then_inc(vsem, 1)
                if t >= 2:
                    mi(t - 2)
            mi(nt - 2)
            mi(nt - 1)

    _nc_cache[key] = nc
    return nc


def run_device(outputs: np.ndarray, trace: bool = False, **kw):
    """Run the SPMD kernel; returns (sumexp [B], pred [B], BassKernelResults)."""
    nc = build_bass()
    xs = outputs.reshape(NCORES, SHARD, C)
    in_maps = [{"x": np.ascontiguousarray(xs[i])} for i in range(NCORES)]
    br = bass_utils.run_bass_kernel_spmd(
        nc, in_maps, list(range(NCORES)), trace=trace, **kw
    )
    res = br.results
    sumexp = np.empty((NCORES, SHARD), np.float64)
    pred = np.empty((NCORES, SHARD), np.int64)
    for i in range(NCORES):
        # staging layout is [partition p, tile t]; flat row index = t*P + p
        sumexp[i] = res[i]["s_out"].astype(np.float64).T.reshape(-1)
        idx0 = res[i]["i_out"].reshape(P, NT, 8)[:, :, 0]
        pred[i] = idx0.T.reshape(-1).astype(np.int64)
    return sumexp.reshape(-1), pred.reshape(-1), br


def finish_host(outputs, targets, cost_matrix, sumexp, pred):
    t = np.asarray(targets).astype(np.int64)
    lse = np.log(sumexp)
    tlogit = outputs[np.arange(B), t].astype(np.float64)
    glp = tlogit - lse

    counts = np.bincount(t * C + pred, minlength=C * C).reshape(C, C)
    cm = cost_matrix.astype(np.float64) + counts
    cm = cm ** 0.25
    np.fill_diagonal(cm, 0.0)
    cm = cm * (BETA2 / cm.max())
    cm = np.clip(cm, BETA1, BETA2)
    gc = cm[t, pred]

    loss = -(glp.mean() * gc.mean())
    return np.asarray(loss, dtype=np.float32)


def kernel(outputs, targets, cost_matrix):
    outputs = np.asarray(outputs)
    sumexp, pred, _ = run_device(outputs)
    return finish_host(outputs, np.asarray(targets), np.asarray(cost_matrix),
                       sumexp, pred)


# revision 20
# speedup vs baseline: 1.0065x; 1.0065x over previous
"""Cost-sensitive cross-entropy loss on 8 Trainium2 NeuronCores.

Strategy (data-parallel over batch):
  - Each of the 8 cores processes a 16384-row shard of `outputs` [131072, 1000].
  - Per 128-row tile the device computes: row-wise sum(exp(x)) (ScalarE Exp with
    accumulate) and row-wise argmax (VectorE max -> max_index, first-occurrence
    semantics matching jnp.argmax).  No max-subtraction is needed: inputs are
    O(1)-scaled so exp() cannot overflow fp32, and skipping it decouples ACT
    from DVE for full engine overlap.
  - Host combines: lse = log(sumexp), glp = x[i,t_i] - lse (gather), scatter-add
    count matrix from (target, predicted), cost-matrix transform, final scalar.
    All host math is O(B + C^2) ~ 1e6 elements vs the 5e8-element device pass.
"""

import contextlib
import sys

import numpy as np

try:
    import concourse.bass as bass
except ImportError:  # stand-alone grading dir: fall back to the repo install
    for p in ("/opt/trn_rl_repo", "/root/.axon_site/_ro/trn_rl_repo"):
        if p not in sys.path:
            sys.path.insert(0, p)
    import concourse.bass as bass

import concourse.mybir as mybir
from concourse import bass_utils

B, C = 131072, 1000
NCORES = 8
SHARD = B // NCORES  # 16384 rows per core
P = 128              # SBUF partitions = rows per tile
NT = SHARD // P      # 128 tiles per core
BETA1, BETA2 = 1.0, 2.0

_nc_cache = {}


def build_bass(nt: int = NT, sx: int = 5, se: int = 8, tpl: int = 4):
    """One-core program; SPMD-replicated across all 8 cores.

    Raw Bass (no Tile): this walrus build allows at most one embedded sync
    wait per instruction, so all waits are standalone wait_ge instructions
    on the issuing engine's stream.

    Loads carry `tpl` 128-row tiles each (2 MB for tpl=4) to amortize the
    ~0.8 us per-DMA-instruction bubble observed on every SDMA engine.

    Pipeline per load L / tile t:
      SP:  [wait exps of slot L-sx done]  dma xt[L%sx] <- x rows
      ACT: [wait load landed; wait argmax(t-se) done]
           ex[t%se] = exp(xt slice), accum -> stage_s[:, t]   (+1 act_sem)
      DVE: [wait exp t done] max8(t) ... max_index(t-2)       (+1 dve_sem)
    """
    key = (nt, sx, se, tpl)
    if key in _nc_cache:
        return _nc_cache[key]

    f32 = mybir.dt.float32
    u32 = mybir.dt.uint32
    Exp = mybir.ActivationFunctionType.Exp
    assert nt % tpl == 0
    nl = nt // tpl  # number of loads

    nc = bass.Bass()
    x = nc.declare_dram_parameter("x", [nt * P, C], f32, isOutput=False)
    s_out = nc.declare_dram_parameter("s_out", [P, nt], f32, isOutput=True)
    i_out = nc.declare_dram_parameter("i_out", [P, nt * 8], u32, isOutput=True)

    with contextlib.ExitStack() as ctx:
        xt = [ctx.enter_context(nc.sbuf_tensor(f"xt{i}", [P, tpl * C], f32))
              for i in range(sx)]
        ex = [ctx.enter_context(nc.sbuf_tensor(f"ex{i}", [P, C], f32))
              for i in range(2)]
        mx8 = [ctx.enter_context(nc.sbuf_tensor(f"mx8_{i}", [P, 8], f32))
               for i in range(8)]
        stage_s = ctx.enter_context(nc.sbuf_tensor("stage_s", [P, nt], f32))
        stage_i = ctx.enter_context(
            nc.sbuf_tensor("stage_i", [P, nt * 8], u32))
        dma_sem = [ctx.enter_context(nc.semaphore(f"dma_sem{i}"))
                   for i in range(sx)]
        out_sem = ctx.enter_context(nc.semaphore("out_sem"))
        act_sem = ctx.enter_context(nc.semaphore("act_sem"))
        dve_sem = ctx.enter_context(nc.semaphore("dve_sem"))
        vsem = ctx.enter_context(nc.semaphore("vsem"))
        block = ctx.enter_context(nc.Block())

        def load(eng, L):
            eng.dma_start(
                out=xt[L % sx][:].rearrange("p (j c) -> p j c", j=tpl),
                in_=x[L * tpl * P:(L + 1) * tpl * P, :].rearrange(
                    "(j p) c -> p j c", p=P),
            ).then_inc(dma_sem[L % sx], 16)

        def slot_free_waits(eng, L):
            # both consumers of slot L-sx done with ALL its tpl tiles
            eng.wait_ge(act_sem, tpl * (L - sx + 1))
            eng.wait_ge(dve_sem, tpl * (L - sx + 1))

        # ACT and DVE each consume xt independently (both gated only on the
        # DMA), so neither compute engine ever waits on the other -- that
        # coupling was serializing exp and max_index before.  One HWDGE
        # queue sustains only ~250 GB/s of descriptor generation, so loads
        # alternate between the SP queue (even L) and the ACT queue (odd L).
        @block.sync
        def _(sync):
            for L in range(0, nl, 2):
                if L >= sx:
                    slot_free_waits(sync, L)
                load(sync, L)
            sync.wait_ge(act_sem, nt)
            sync.dma_start(out=s_out[:, :], in_=stage_s[:]).then_inc(out_sem, 16)
            sync.wait_ge(dve_sem, nt)
            sync.dma_start(out=i_out[:, :], in_=stage_i[:]).then_inc(out_sem, 16)
            sync.wait_ge(out_sem, 32)

        @block.scalar
        def _(scalar):
            # odd load L fires after exp tile tpl*(L-sx+1)+3: its guard sems
            # passed that value >=2 tiles ago on both engines (no stall),
            # and the data isn't needed until tile L*tpl, ~2 loads later.
            trig = {}
            for L in range(1, nl, 2):
                t_fire = max(tpl * (L - sx + 1) + 3, 0)
                trig.setdefault(t_fire, []).append(L)
            for L in trig.get(0, []):
                load(scalar, L)
            for t in range(nt):
                L, j = divmod(t, tpl)
                if j == 0:
                    scalar.wait_ge(dma_sem[L % sx], 16 * (L // sx + 1))
                if t >= 2:
                    # exp(t-2) done -> dummy out slot t%2 free (own stream,
                    # always satisfied: exp(t-1) is in flight, t-2 retired)
                    scalar.wait_ge(act_sem, t - 1)
                scalar.activation(
                    ex[t % 2][:], xt[L % sx][:, j * C:(j + 1) * C], Exp,
                    accum_out=stage_s[:, t:t + 1],
                ).then_inc(act_sem, 1)
                for Lq in trig.get(t + 1, []):
                    scalar.wait_ge(act_sem, tpl * (Lq - sx + 1))
                    scalar.wait_ge(dve_sem, tpl * (Lq - sx + 1))
                    load(scalar, Lq)

        @block.vector
        def _(vector):
            # 2-stage SW pipeline: max(t) runs 2 tiles ahead of
            # max_index(t-2), so every sem wait (RAW on mx8, WAR on mx8
            # slot recycle) is satisfied well before it executes -- no DVE
            # pipeline drain.  Reads raw x: argmax(x) == argmax(exp(x)).
            def xsl(t):
                L, j = divmod(t, tpl)
                return xt[L % sx][:, j * C:(j + 1) * C]

            def mi(t):
                vector.wait_ge(vsem, t + 1)
                vector.max_index(
                    stage_i[:, t * 8:(t + 1) * 8], mx8[t % 8][:], xsl(t),
                ).then_inc(dve_sem, 1)

            for t in range(nt):
                L, j = divmod(t, tpl)
                if j == 0:
                    vector.wait_ge(dma_sem[L % sx], 16 * (L // sx + 1))
                if t >= 8:
                    vector.wait_ge(dve_sem, t - 7)  # mi(t-8) freed mx8 slot
                vector.max(mx8[t % 8][:], xsl(t)).then_inc(vsem, 1)
                if t >= 2:
                    mi(t - 2)
            mi(nt - 2)
            mi(nt - 1)

    _nc_cache[key] = nc
    return nc


def run_device(outputs: np.ndarray, trace: bool = False, **kw):
    """Run the SPMD kernel; returns (sumexp [B], pred [B], BassKernelResults)."""
    nc = build_bass()
    xs = outputs.reshape(NCORES, SHARD, C)
    in_maps = [{"x": np.ascontiguousarray(xs[i])} for i in range(NCORES)]
    br = bass_utils.run_bass_kernel_spmd(
        nc, in_maps, list(range(NCORES)), trace=trace, **kw
    )
    res = br.results
    sumexp = np.empty((NCORES, SHARD), np.float64)
    pred = np.empty((NCORES, SHARD), np.int64)
    for i in range(NCORES):
        # staging layout is [partition p, tile t]; flat row index = t*P + p
        sumexp[i] = res[i]["s_out"].astype(np.float64).T.reshape(-1)
        idx0 = res[i]["i_out"].reshape(P, NT, 8)[:, :, 0]
        pred[i] = idx0.T.reshape(-1).astype(np.int64)
    return sumexp.reshape(-1), pred.reshape(-1), br


def finish_host(outputs, targets, cost_matrix, sumexp, pred):
    t = np.asarray(targets).astype(np.int64)
    lse = np.log(sumexp)
    tlogit = outputs[np.arange(B), t].astype(np.float64)
    glp = tlogit - lse

    counts = np.bincount(t * C + pred, minlength=C * C).reshape(C, C)
    cm = cost_matrix.astype(np.float64) + counts
    cm = cm ** 0.25
    np.fill_diagonal(cm, 0.0)
    cm = cm * (BETA2 / cm.max())
    cm = np.clip(cm, BETA1, BETA2)
    gc = cm[t, pred]

    loss = -(glp.mean() * gc.mean())
    return np.asarray(loss, dtype=np.float32)


def kernel(outputs, targets, cost_matrix):
    outputs = np.asarray(outputs)
    sumexp, pred, _ = run_device(outputs)
    return finish_host(outputs, np.asarray(targets), np.asarray(cost_matrix),
                       sumexp, pred)


# revision 21
# speedup vs baseline: 1.0122x; 1.0056x over previous
"""Cost-sensitive cross-entropy loss on 8 Trainium2 NeuronCores.

Strategy (data-parallel over batch):
  - Each of the 8 cores processes a 16384-row shard of `outputs` [131072, 1000].
  - Per 128-row tile the device computes: row-wise sum(exp(x)) (ScalarE Exp with
    accumulate) and row-wise argmax (VectorE max -> max_index, first-occurrence
    semantics matching jnp.argmax).  No max-subtraction is needed: inputs are
    O(1)-scaled so exp() cannot overflow fp32, and skipping it decouples ACT
    from DVE for full engine overlap.
  - Host combines: lse = log(sumexp), glp = x[i,t_i] - lse (gather), scatter-add
    count matrix from (target, predicted), cost-matrix transform, final scalar.
    All host math is O(B + C^2) ~ 1e6 elements vs the 5e8-element device pass.
"""

import contextlib
import sys

import numpy as np

try:
    import concourse.bass as bass
except ImportError:  # stand-alone grading dir: fall back to the repo install
    for p in ("/opt/trn_rl_repo", "/root/.axon_site/_ro/trn_rl_repo"):
        if p not in sys.path:
            sys.path.insert(0, p)
    import concourse.bass as bass

import concourse.mybir as mybir
from concourse import bass_utils

B, C = 131072, 1000
NCORES = 8
SHARD = B // NCORES  # 16384 rows per core
P = 128              # SBUF partitions = rows per tile
NT = SHARD // P      # 128 tiles per core
BETA1, BETA2 = 1.0, 2.0

_nc_cache = {}


def build_bass(nt: int = NT, sx: int = 8, se: int = 8, tpl: int = 4):
    """One-core program; SPMD-replicated across all 8 cores.

    Raw Bass (no Tile): this walrus build allows at most one embedded sync
    wait per instruction, so all waits are standalone wait_ge instructions
    on the issuing engine's stream.

    Loads carry `tpl` 128-row tiles each (2 MB for tpl=4) to amortize the
    ~0.8 us per-DMA-instruction bubble observed on every SDMA engine.

    Pipeline per load L / tile t:
      SP:  [wait exps of slot L-sx done]  dma xt[L%sx] <- x rows
      ACT: [wait load landed; wait argmax(t-se) done]
           ex[t%se] = exp(xt slice), accum -> stage_s[:, t]   (+1 act_sem)
      DVE: [wait exp t done] max8(t) ... max_index(t-2)       (+1 dve_sem)
    """
    key = (nt, sx, se, tpl)
    if key in _nc_cache:
        return _nc_cache[key]

    f32 = mybir.dt.float32
    u32 = mybir.dt.uint32
    Exp = mybir.ActivationFunctionType.Exp
    assert nt % tpl == 0
    nl = nt // tpl  # number of loads

    nc = bass.Bass()
    x = nc.declare_dram_parameter("x", [nt * P, C], f32, isOutput=False)
    s_out = nc.declare_dram_parameter("s_out", [P, nt], f32, isOutput=True)
    i_out = nc.declare_dram_parameter("i_out", [P, nt * 8], u32, isOutput=True)

    with contextlib.ExitStack() as ctx:
        xt = [ctx.enter_context(nc.sbuf_tensor(f"xt{i}", [P, tpl * C], f32))
              for i in range(sx)]
        ex = [ctx.enter_context(nc.sbuf_tensor(f"ex{i}", [P, C], f32))
              for i in range(2)]
        mx8 = [ctx.enter_context(nc.sbuf_tensor(f"mx8_{i}", [P, 8], f32))
               for i in range(8)]
        stage_s = ctx.enter_context(nc.sbuf_tensor("stage_s", [P, nt], f32))
        stage_i = ctx.enter_context(
            nc.sbuf_tensor("stage_i", [P, nt * 8], u32))
        dma_sem = [ctx.enter_context(nc.semaphore(f"dma_sem{i}"))
                   for i in range(sx)]
        out_sem = ctx.enter_context(nc.semaphore("out_sem"))
        act_sem = ctx.enter_context(nc.semaphore("act_sem"))
        dve_sem = ctx.enter_context(nc.semaphore("dve_sem"))
        vsem = ctx.enter_context(nc.semaphore("vsem"))
        block = ctx.enter_context(nc.Block())

        def load(eng, L):
            eng.dma_start(
                out=xt[L % sx][:].rearrange("p (j c) -> p j c", j=tpl),
                in_=x[L * tpl * P:(L + 1) * tpl * P, :].rearrange(
                    "(j p) c -> p j c", p=P),
            ).then_inc(dma_sem[L % sx], 16)

        def slot_free_waits(eng, L):
            # both consumers of slot L-sx done with ALL its tpl tiles
            eng.wait_ge(act_sem, tpl * (L - sx + 1))
            eng.wait_ge(dve_sem, tpl * (L - sx + 1))

        # ACT and DVE each consume xt independently (both gated only on the
        # DMA), so neither compute engine ever waits on the other -- that
        # coupling was serializing exp and max_index before.  One HWDGE
        # queue sustains only ~250 GB/s of descriptor generation, so loads
        # alternate between the SP queue (even L) and the ACT queue (odd L).
        @block.sync
        def _(sync):
            for L in range(0, nl, 2):
                if L >= sx:
                    slot_free_waits(sync, L)
                load(sync, L)
            sync.wait_ge(act_sem, nt)
            sync.dma_start(out=s_out[:, :], in_=stage_s[:]).then_inc(out_sem, 16)
            sync.wait_ge(dve_sem, nt)
            sync.dma_start(out=i_out[:, :], in_=stage_i[:]).then_inc(out_sem, 16)
            sync.wait_ge(out_sem, 32)

        @block.scalar
        def _(scalar):
            # odd load L fires after exp tile tpl*(L-sx+1)+3: its guard sems
            # passed that value >=2 tiles ago on both engines (no stall),
            # and the data isn't needed until tile L*tpl, ~2 loads later.
            trig = {}
            for L in range(1, nl, 2):
                t_fire = max(tpl * (L - sx) + 9, 0)
                trig.setdefault(t_fire, []).append(L)
            for L in trig.get(0, []):
                load(scalar, L)
            for t in range(nt):
                L, j = divmod(t, tpl)
                if j == 0:
                    scalar.wait_ge(dma_sem[L % sx], 16 * (L // sx + 1))
                if t >= 2:
                    # exp(t-2) done -> dummy out slot t%2 free (own stream,
                    # always satisfied: exp(t-1) is in flight, t-2 retired)
                    scalar.wait_ge(act_sem, t - 1)
                scalar.activation(
                    ex[t % 2][:], xt[L % sx][:, j * C:(j + 1) * C], Exp,
                    accum_out=stage_s[:, t:t + 1],
                ).then_inc(act_sem, 1)
                for Lq in trig.get(t + 1, []):
                    scalar.wait_ge(act_sem, tpl * (Lq - sx + 1))
                    scalar.wait_ge(dve_sem, tpl * (Lq - sx + 1))
                    load(scalar, Lq)

        @block.vector
        def _(vector):
            # 2-stage SW pipeline: max(t) runs 2 tiles ahead of
            # max_index(t-2), so every sem wait (RAW on mx8, WAR on mx8
            # slot recycle) is satisfied well before it executes -- no DVE
            # pipeline drain.  Reads raw x: argmax(x) == argmax(exp(x)).
            def xsl(t):
                L, j = divmod(t, tpl)
                return xt[L % sx][:, j * C:(j + 1) * C]

            def mi(t):
                vector.wait_ge(vsem, t + 1)
                vector.max_index(
                    stage_i[:, t * 8:(t + 1) * 8], mx8[t % 8][:], xsl(t),
                ).then_inc(dve_sem, 1)

            for t in range(nt):
                L, j = divmod(t, tpl)
                if j == 0:
                    vector.wait_ge(dma_sem[L % sx], 16 * (L // sx + 1))
                if t >= 8:
                    vector.wait_ge(dve_sem, t - 7)  # mi(t-8) freed mx8 slot
                vector.max(mx8[t % 8][:], xsl(t)).then_inc(vsem, 1)
                if t >= 2:
                    mi(t - 2)
            mi(nt - 2)
            mi(nt - 1)

    _nc_cache[key] = nc
    return nc


def run_device(outputs: np.ndarray, trace: bool = False, **kw):
    """Run the SPMD kernel; returns (sumexp [B], pred [B], BassKernelResults)."""
    nc = build_bass()
    xs = outputs.reshape(NCORES, SHARD, C)
    in_maps = [{"x": np.ascontiguousarray(xs[i])} for i in range(NCORES)]
    br = bass_utils.run_bass_kernel_spmd(
        nc, in_maps, list(range(NCORES)), trace=trace, **kw
    )
    res = br.results
    sumexp = np.empty((NCORES, SHARD), np.float64)
    pred = np.empty((NCORES, SHARD), np.int64)
    for i in range(NCORES):
        # staging layout is [partition p, tile t]; flat row index = t*P + p
        sumexp[i] = res[i]["s_out"].astype(np.float64).T.reshape(-1)
        idx0 = res[i]["i_out"].reshape(P, NT, 8)[:, :, 0]
        pred[i] = idx0.T.reshape(-1).astype(np.int64)
    return sumexp.reshape(-1), pred.reshape(-1), br


def finish_host(outputs, targets, cost_matrix, sumexp, pred):
    t = np.asarray(targets).astype(np.int64)
    lse = np.log(sumexp)
    tlogit = outputs[np.arange(B), t].astype(np.float64)
    glp = tlogit - lse

    counts = np.bincount(t * C + pred, minlength=C * C).reshape(C, C)
    cm = cost_matrix.astype(np.float64) + counts
    cm = cm ** 0.25
    np.fill_diagonal(cm, 0.0)
    cm = cm * (BETA2 / cm.max())
    cm = np.clip(cm, BETA1, BETA2)
    gc = cm[t, pred]

    loss = -(glp.mean() * gc.mean())
    return np.asarray(loss, dtype=np.float32)


def kernel(outputs, targets, cost_matrix):
    outputs = np.asarray(outputs)
    sumexp, pred, _ = run_device(outputs)
    return finish_host(outputs, np.asarray(targets), np.asarray(cost_matrix),
                       sumexp, pred)
